# revision 27
# baseline (speedup 1.0000x reference)
"""BibdLinear Trainium2 kernel: out = input @ (weight * mask).T

Shapes (hardcoded): input [8192, 4096] f32, weight [4096, 4096] f32,
mask [4096, 4096] f32 -> out [8192, 4096] f32.

Sharding (column-parallel x batch-parallel, 8 cores):
  2 batch shards x 4 output-feature shards. Core c handles batch rows
  [(c//4)*4096, +4096) and output features [(c%4)*1024, +1024). The host
  folds the (static, 0/1) BIBD mask into the weight layout (wm = w*mask,
  bf16) and feeds contraction-major slices; each core runs a dense GEMM
  on the tensor engine; the host concatenates the 8 output slices.

Per-core device program (Bass/Tile):
  - inputs: xL [2048, 8192] bf16 (x pre-tiled on host to
    [block, k-partition, k-tile*256] so each 256-row batch block is one
    fully-contiguous 2 MB region) and wmT [4096, 1024] bf16 (bf16 matmul
    runs at the same 1 cycle/row rate as float32r but halves DMA traffic;
    rms error ~2e-3 vs the f32 reference, well under the 2e-2 gate).
  - wmT is DMA'd once into 32 resident SBUF k-strips [128, 1024] on the
    ACT queue; x streams as two contiguous 1 MB DMAs per batch block
    (~341 GB/s vs ~138 GB/s for the per-k 64 KB strided tiles a [K, B]
    layout would need) on the SP queue; output stores ride the ACT queue.
  - GEMM: per 256-row batch block, accumulate over 32 k-tiles into PSUM;
    lhsT = x k-tile [128, 128] (stationary, 1 LDWEIGHTS per 2 matmuls,
    hidden by the PE background weight buffer + FWL), rhs = wm chunk
    [128, 512] (moving). Key rate fact (HW-measured): the PE moving
    operand streams 32 bits/lane/cycle, so bf16 feeds TWO columns per
    cycle — an N=512 bf16 matmul costs ~256 cycles (~109 ns), half the
    f32r cost per output column. This is why the moving-operand max is
    1024 for bf16 vs 512 for fp32.
  - PSUM: 4 accumulation groups [128, 512] f32 per block = one full bank
    each (a start=True matmul clears its whole bank, so groups can never
    share one), double-buffered (bufs=2 -> all 8 banks): block b+1
    accumulates into the other 4-bank set while block b's banks are
    evicted, split across DVE (subtile 0) and ACT (subtile 1).
  - output is stored bf16 (halves store traffic; rms error contribution
    ~0.2%) and upcast to f32 on the host.
"""

import numpy as np
import ml_dtypes

import concourse.mybir as mybir
import concourse.tile as tile
from concourse import bacc
from concourse.bass_utils import run_bass_kernel_spmd

BATCH, IN_F, OUT_F = 8192, 4096, 4096
B_S, O_S = 2, 4                      # batch shards x out-feature shards
B, OF = BATCH // B_S, OUT_F // O_S   # 4096, 1024 per core
N_CORES = 8

NB = 256   # batch block width (2 subtiles of 128)
NF = 512   # moving (feature) chunk width per matmul

F32 = mybir.dt.float32
BF16 = mybir.dt.bfloat16

_NC_CACHE = {}


def _build_nc(x_bufs=3, out_bufs=4, iters=1,
              nblk=None, wm_once=False, skip_x=False, skip_out=False,
              skip_mm=False, nf=None, psum_bufs=2, out_bf16=True):
    """nblk/wm_once/skip_x/skip_out/skip_mm/nf/psum_bufs are timing-
    ablation knobs for dev experiments; the graded kernel uses the
    defaults."""
    NF_ = nf if nf is not None else NF
    K = IN_F
    KO = K // 128          # 32 contraction tiles
    B_SUB = NB // 128      # 2 batch subtiles per block
    OC = OF // NF_         # feature chunks
    NBLK = nblk if nblk is not None else B // NB   # 16 batch blocks

    nc = bacc.Bacc(None, target_bir_lowering=False)

    ODT = BF16 if out_bf16 else F32
    xL = nc.dram_tensor("xL", [NBLK * 128, KO * NB], BF16,
                        kind="ExternalInput")
    wT = nc.dram_tensor("wT", [K, OF], BF16, kind="ExternalInput")
    out = nc.dram_tensor("out", [B, OF], ODT, kind="ExternalOutput")

    xL3 = xL.rearrange("(bb p) c -> bb p c", p=128)
    wT3 = wT.rearrange("(ko p) o -> ko p o", p=128)
    KHALF = KO // 2

    with tile.TileContext(nc) as tc:
        with (
            tc.tile_pool(name="wpool", bufs=1) as wpool,
            tc.tile_pool(name="xpool", bufs=x_bufs) as xpool,
            tc.tile_pool(name="opool", bufs=out_bufs) as opool,
            tc.tile_pool(name="psum", bufs=psum_bufs, space="PSUM") as psum_pool,
        ):
            wm = [None] * KO

            def load_wm():
                for k in range(KO):
                    wt = wpool.tile([128, OF], BF16, tag=f"wm{k}",
                                    name=f"wm{k}")
                    nc.scalar.dma_start(wt, wT3[k])
                    wm[k] = wt

            def body():
                # Warm-up: ~12 throwaway matmuls on a zeroed tile (sourced
                # from the pre-zeroed output buffer) fill the PE's dead
                # window while x chunk 0 / wm strip 0 are in flight and
                # ramp the HAM clock gate to 8/8 before real work starts.
                # Their psum garbage is wiped by block 0's start=True.
                if out_bf16 and not skip_mm:
                    wrm = xpool.tile([128, KHALF * NB], BF16, tag="xt0",
                                     name="warm_x")
                    nc.sync.dma_start(wrm[:, :NF_], out[0:128, 0:NF_])
                    pw = psum_pool.tile([128, NF_], F32, tag="ps0",
                                        name="warm_ps")
                    for i in range(12):
                        nc.tensor.matmul(pw, wrm[:, 0:128], wrm[:, 0:NF_],
                                         start=(i == 0), stop=(i == 11))
                if not wm_once:
                    load_wm()

                for bb in range(NBLK):
                    psums = [
                        psum_pool.tile([128, NF_], F32, tag=f"ps{i}",
                                       name=f"ps{i}_{bb}")
                        for i in range(B_SUB * OC)
                    ]
                    # two half-block x tiles -> k=0 matmuls start after the
                    # first contiguous 1 MB DMA, not the full 2 MB block
                    xts = []
                    for h in range(2):
                        xt = xpool.tile([128, KHALF * NB], BF16, tag=f"xt{h}",
                                        name=f"xt{bb}_{h}")
                        if not skip_x:
                            nc.sync.dma_start(
                                xt,
                                xL3[bb, :, h * KHALF * NB:(h + 1) * KHALF * NB]
                            )
                        xts.append(xt)
                    if skip_mm:
                        continue
                    for k in range(KO):
                        xt = xts[k // KHALF]
                        kc = (k % KHALF) * NB
                        for bs in range(B_SUB):
                            lhsT = xt[:, kc + bs * 128:kc + (bs + 1) * 128]
                            for oc in range(OC):
                                nc.tensor.matmul(
                                    psums[bs * OC + oc], lhsT,
                                    wm[k][:, oc * NF_:(oc + 1) * NF_],
                                    start=(k == 0), stop=(k == KO - 1),
                                )
                    if skip_out:
                        continue
                    ots = [
                        opool.tile([128, OF], ODT, tag=f"ot{bs}",
                                   name=f"ot{bb}_{bs}")
                        for bs in range(B_SUB)
                    ]
                    # Evict bs0 on DVE and bs1 on ACT concurrently; ACT's
                    # copies are issued before its store DMAs so its queue
                    # never blocks waiting on DVE.
                    for oc in range(OC):
                        nc.scalar.copy(ots[1][:, oc * NF_:(oc + 1) * NF_],
                                       psums[OC + oc])
                    for oc in range(OC):
                        nc.vector.tensor_copy(
                            ots[0][:, oc * NF_:(oc + 1) * NF_], psums[oc])
                    for bs in range(B_SUB):
                        nc.scalar.dma_start(
                            out[bb * NB + bs * 128: bb * NB + (bs + 1) * 128,
                                :], ots[bs]
                        )

            if wm_once:
                load_wm()
            if iters == 1:
                body()
            else:
                with tc.For_i(0, iters):
                    body()

    nc.compile()
    return nc


def _get_nc():
    if "nc" not in _NC_CACHE:
        _NC_CACHE["nc"] = _build_nc()
    return _NC_CACHE["nc"]


def shard_inputs(input, weight, mask):
    """Host-side sharding/layout: per-core contraction-major bf16 slices
    with the static BIBD mask folded into the weight."""
    bf16 = ml_dtypes.bfloat16
    x = np.asarray(input, dtype=np.float32).astype(bf16)
    wm = (np.asarray(weight, dtype=np.float32)
          * np.asarray(mask, dtype=np.float32))
    NBLK = B // NB
    KO = IN_F // 128
    # xL[bb*128+p, ko*NB+c] = x[b0+bb*NB+c, ko*128+p]: one contiguous 2 MB
    # region per batch block. One layout pass per batch half, shared by 4
    # cores each.
    xL_half = [
        np.ascontiguousarray(
            x[h * B:(h + 1) * B, :]
            .reshape(NBLK, NB, KO, 128)
            .transpose(0, 3, 2, 1)
            .reshape(NBLK * 128, KO * NB)
        )
        for h in range(B_S)
    ]
    in_maps = []
    for c in range(N_CORES):
        o0 = (c % O_S) * OF
        in_maps.append({
            "xL": xL_half[c // O_S],
            "wT": np.ascontiguousarray(wm[o0:o0 + OF, :].T).astype(bf16),
        })
    return in_maps


def gather_output(results):
    outp = np.empty((BATCH, OUT_F), np.float32)
    for c in range(N_CORES):
        b0 = (c // O_S) * B
        o0 = (c % O_S) * OF
        outp[b0:b0 + B, o0:o0 + OF] = results[c]["out"].astype(np.float32)
    return outp


def kernel(input, weight, mask):
    in_maps = shard_inputs(input, weight, mask)
    res = run_bass_kernel_spmd(_get_nc(), in_maps, core_ids=list(range(N_CORES)))
    return gather_output(res.results)
